# revision 20
# baseline (speedup 1.0000x reference)
"""Trainium2 Bass kernel for the MFCA channel-attention module.

  q = x_RGB.reshape(B, C, N); k = v = x.reshape(B, C, N)
  energy    = q @ k^T                          (B, C, C)
  attention = softmax(max(energy, -1) - energy)   over last axis
  out       = delta * (attention @ v) + x

Numerically, softmax(max - energy) == softmax(-energy); the stable form is
p = exp(min_row(energy) - energy), attention = p / rowsum(p).

Sharding: data-parallel over batch B=16 across 8 NeuronCores (2 per core).

v5 (all-fp8 matmul path + deep load run-ahead):
  - both big matmuls run fp8e4 with perf_mode=DoubleRow (2 MACs/cell/cycle,
    256-deep contraction per instruction).
  - Q is cast-DMA'd straight to fp8 [128, ct, n] (SWDGE fp32->fp8, validated
    exact); x is cast-DMA'd to bf16 quarters (residual) and engine-cast to
    fp8 on ACT (V for MM2 + K-transpose source).  Small SBUF footprints let
    the next batch's loads run far ahead, keeping the DMA queues fed — the
    dominant stall in earlier revisions.
  - Q^T/K^T are fp8 PE transposes (4x LDWEIGHTS via FWL).  fp8 transpose
    mode must write PSUM with element step 2, so stages are [128,2,C,2] and
    drains read the strided [...,0] view, writing the fp8 pair layout
    [128, 2(chunk), 2(q/k), C] whose slices are the DoubleRow APs.
  - MM2 blocks of the PREVIOUS batch (always data-ready) are interleaved
    2-per-MM1-pair into the front so the in-order PE queue never
    head-of-line-blocks on a load; stores are merged per (i, n-half).
"""

from contextlib import ExitStack

import numpy as np

import concourse.bass as bass
import concourse.tile as tile
from concourse import bacc, mybir
from concourse.bass_utils import run_bass_kernel_spmd
from concourse.masks import make_identity

N_CORES = 8
B, C, H, W = 16, 512, 64, 64
N = H * W  # 4096
BS = B // N_CORES  # batches per core

F32 = mybir.dt.float32
BF16 = mybir.dt.bfloat16
FP8 = mybir.dt.float8e4

DR = mybir.MatmulPerfMode.DoubleRow


def build_nc(bs=BS, c=C, n=N):
    """Build the single-core Bass program (SPMD across all cores)."""
    nc = bacc.Bacc(None, target_bir_lowering=False, debug=False)

    x_d = nc.dram_tensor("x", [bs, c, n], F32, kind="ExternalInput")
    q_d = nc.dram_tensor("x_RGB", [bs, c, n], F32, kind="ExternalInput")
    d_d = nc.dram_tensor("delta", [128, 1], F32, kind="ExternalInput")
    o_d = nc.dram_tensor("out", [bs, c, n], BF16, kind="ExternalOutput")

    nct = c // 128  # channel chunks (i-tiles / j-tiles)
    nnt = n // 128  # n-chunks in the energy contraction
    npr = nnt // 2  # DoubleRow n-pairs
    nnb = n // 512  # n-blocks in the output matmul
    half = n // 4  # 1024 (load quarter)
    nh = 4

    with tile.TileContext(nc) as tc, ExitStack() as ctx:
        pxb = ctx.enter_context(tc.tile_pool(name="pxb", bufs=34))
        px8 = ctx.enter_context(tc.tile_pool(name="px8", bufs=2))
        pq8 = ctx.enter_context(tc.tile_pool(name="pq8", bufs=2))
        pqt = ctx.enter_context(tc.tile_pool(name="pqt", bufs=6))
        pp = ctx.enter_context(tc.tile_pool(name="pp", bufs=6))
        ppt = ctx.enter_context(tc.tile_pool(name="ppt", bufs=5))
        pout = ctx.enter_context(tc.tile_pool(name="pout", bufs=8))
        psml = ctx.enter_context(tc.tile_pool(name="psml", bufs=8))
        pone = ctx.enter_context(tc.tile_pool(name="pone", bufs=1))
        pe_pool = ctx.enter_context(tc.tile_pool(name="pe", bufs=4, space="PSUM"))
        ptr_pool = ctx.enter_context(tc.tile_pool(name="ptr", bufs=2, space="PSUM"))
        pu_pool = ctx.enter_context(tc.tile_pool(name="pu", bufs=2, space="PSUM"))

        ident8 = pone.tile([128, 128], FP8)
        make_identity(nc, ident8[:])
        delta_sb = pone.tile([128, 1], F32)
        nc.sync.dma_start(out=delta_sb[:], in_=d_d[:])

        def emit_loads(b):
            """Load one batch: bf16 x quarters (residual), fp8 x_RGB direct,
            fp8 x via ACT cast (V + K-transpose source).

            n-quarter major order so the transpose pipeline (which consumes
            all channel chunks of one n-range at a time) starts earliest."""
            xbs = [[None] * nh for _ in range(nct)]
            x8 = px8.tile([128, nct, n], FP8, name="x8", tag="x8")
            q8 = pq8.tile([128, nct, n], FP8, name="q8", tag="q8")

            for h in range(nh):
                cs = slice(h * half, (h + 1) * half)
                for k in range(nct):
                    xb = pxb.tile([128, half], BF16)
                    nc.gpsimd.dma_start(
                        out=xb[:], in_=x_d[b, 128 * k : 128 * (k + 1), cs]
                    )
                    nc.gpsimd.dma_start(
                        out=q8[:, k, cs], in_=q_d[b, 128 * k : 128 * (k + 1), cs]
                    )
                    nc.scalar.copy(out=x8[:, k, cs], in_=xb[:])
                    xbs[k][h] = xb
            return xbs, x8, q8

        def _sl(tiles, cc, c0, w):
            """Slice [c0, c0+w) of chunk cc out of per-quarter tiles."""
            h = c0 // half
            return tiles[cc][h][:, c0 - h * half : c0 - h * half + w]

        def drain_eng(l):
            return "v" if l % 2 == 0 else "s"  # 16 DVE / 16 ACT per batch

        def t_stream(p, nxt, q8, x8):
            """Generator yielding after each fp8 PE transpose of pair p, so
            the caller can interleave them with MM1 matmuls.  Both chunks of
            the pair share ONE stage bank: fp8 transpose mode writes at
            element step 2, so chunk li lands in the [..., li] byte lane —
            one PSUM bank now holds a full pair, doubling stage depth."""
            for li in range(2):
                nt = 2 * p + li
                ns = slice(128 * nt, 128 * (nt + 1))
                stage = ptr_pool.tile(
                    [128, 2, c, 2], FP8, name="tstage", tag="stage"
                )
                for cc in range(nct):
                    nc.tensor.transpose(
                        stage[:, 0, 128 * cc : 128 * (cc + 1), 0],
                        q8[:, cc, ns],
                        ident8[:],
                    )
                    yield
                    nc.tensor.transpose(
                        stage[:, 1, 128 * cc : 128 * (cc + 1), 0],
                        x8[:, cc, ns],
                        ident8[:],
                    )
                    yield
                if drain_eng(nt) == "v":
                    nc.vector.tensor_copy(out=nxt[:, li, :, :], in_=stage[:, :, :, 0])
                else:
                    nc.scalar.copy(out=nxt[:, li, :, :], in_=stage[:, :, :, 0])

        def emit_transpose_pair(p, q8, x8):
            qxt = pqt.tile([128, 2, 2, c], FP8)
            for _ in t_stream(p, qxt, q8, x8):
                pass
            return qxt

        def emit_mm1_pair(p, es, qxt, ts):
            """4 DoubleRow matmuls (one per i-tile) for n-pair p, interleaving
            the next pair's transposes 4-per-matmul."""
            for i in range(nct):
                nc.tensor.matmul(
                    es[i][:],
                    qxt[:, :, 0, 128 * i : 128 * (i + 1)],
                    qxt[:, :, 1, :],
                    start=(p == 0),
                    stop=(p == npr - 1),
                    perf_mode=DR,
                )
                if ts is not None:
                    for _ in range(4):
                        next(ts, None)
            if ts is not None:
                for _ in ts:
                    pass

        def emit_softmax(i, es):
            e = es[i]
            m = psml.tile([128, 1], F32)
            nc.vector.tensor_reduce(
                m[:], e[:], axis=mybir.AxisListType.X, op=mybir.AluOpType.min
            )
            p_t = pp.tile([128, c], BF16)
            z = psml.tile([128, 1], F32)
            nc.scalar.activation(
                out=p_t[:],
                in_=e[:],
                func=mybir.ActivationFunctionType.Exp,
                bias=m[:],
                scale=-1.0,
                accum_out=z[:],
            )
            zi = psml.tile([128, 1], F32)
            nc.vector.reciprocal(zi[:], z[:])
            s = psml.tile([128, 1], F32)
            nc.vector.tensor_scalar_mul(s[:], zi[:], delta_sb[:])  # delta / Z
            # Fold delta/Z into P here so MM2 needs no per-block scaling.
            ps = pp.tile([128, c], FP8)
            nc.vector.tensor_scalar_mul(ps[:], p_t[:], s[:])
            # P'^T via fp8 PE transposes (step-2 stage), drained to
            # [128, jt, 128] so the MM2 DoubleRow stationary is a jt-pair
            # slice.
            pstage = ptr_pool.tile(
                [128, nct, 128, 2], FP8, name="pstage", tag="stage"
            )
            for jt in range(nct):
                nc.tensor.transpose(
                    pstage[:, jt, :, 0],
                    ps[:, 128 * jt : 128 * (jt + 1)],
                    ident8[:],
                )
            pt = ppt.tile([128, nct, 128], FP8)
            nc.scalar.copy(out=pt[:], in_=pstage[:, :, :, 0])
            return pt

        def mm2_stream(b, sm, x8, xbs):
            """Generator of MM2 blocks: one (i, nb) output block per step;
            epilogue adds bf16 x and merges stores per (i, n-half)."""
            for hf in range(2):
                obufs = [
                    pout.tile([128, n // 2], BF16, name=f"ob{hf}_{i}", tag="ob")
                    for i in range(nct)
                ]
                for nb in range(nnb // 2):
                    gnb = hf * (nnb // 2) + nb
                    ns = slice(512 * gnb, 512 * (gnb + 1))
                    for i in range(nct):
                        pt = sm[i]
                        u = pu_pool.tile([128, 512], F32, name="u", tag="u")
                        for jp in range(2):
                            nc.tensor.matmul(
                                u[:],
                                pt[:, 2 * jp : 2 * jp + 2, :],
                                x8[:, 2 * jp : 2 * jp + 2, ns],
                                start=(jp == 0),
                                stop=(jp == 1),
                                perf_mode=DR,
                            )
                        nc.vector.tensor_add(
                            obufs[i][:, 512 * nb : 512 * (nb + 1)],
                            u[:],
                            _sl(xbs, i, 512 * gnb, 512),
                        )
                        yield
                hs = slice(hf * (n // 2), (hf + 1) * (n // 2))
                for i in range(nct):
                    nc.sync.dma_start(
                        out=o_d[b, 128 * i : 128 * (i + 1), hs], in_=obufs[i][:]
                    )

        def emit_batch_front(b, mm2):
            """Loads, transposes, energy matmuls, and softmax for one batch;
            interleaves the previous batch's MM2 blocks (always data-ready)
            2-per-pair so the PE never head-of-line-blocks on load DMAs."""
            xbs, x8, q8 = emit_loads(b)
            es = [
                pe_pool.tile([128, c], F32, name=f"e{i}", tag="e") for i in range(nct)
            ]
            depth = 3
            pairs = [None] * depth
            for p0 in range(depth):
                pairs[p0] = emit_transpose_pair(p0, q8, x8)
            for p in range(npr):
                qxt = pairs[p % depth]
                ts = None
                if p + depth < npr:
                    nxt = pqt.tile([128, 2, 2, c], FP8, name="qxt", tag="qxt")
                    ts = t_stream(p + depth, nxt, q8, x8)
                    pairs[p % depth] = nxt
                emit_mm1_pair(p, es, qxt, ts)
                if mm2 is not None:
                    next(mm2, None)
            # reserve the remaining MM2 blocks to fill the PE while the
            # DVE/ACT softmax chain runs
            sm = []
            for i in range(nct):
                sm.append(emit_softmax(i, es))
                if mm2 is not None:
                    for _ in range(4):
                        next(mm2, None)
            if mm2 is not None:
                for _ in mm2:
                    pass
            return xbs, x8, sm

        mm2 = None
        for b in range(bs):
            xbs, x8, sm = emit_batch_front(b, mm2)
            mm2 = mm2_stream(b, sm, x8, xbs)
        for _ in mm2:
            pass

    nc.compile()
    return nc


_NC_CACHE = {}


def _get_nc(key=(BS, C, N)):
    if key not in _NC_CACHE:
        _NC_CACHE[key] = build_nc(*key)
    return _NC_CACHE[key]


def _run(x, x_RGB, delta, trace=False):
    x = np.ascontiguousarray(np.asarray(x, dtype=np.float32)).reshape(B, C, N)
    xr = np.ascontiguousarray(np.asarray(x_RGB, dtype=np.float32)).reshape(B, C, N)
    d = np.asarray(delta, dtype=np.float32).reshape(-1)[0]
    d_b = np.full((128, 1), d, dtype=np.float32)

    nc = _get_nc()
    in_maps = []
    for cid in range(N_CORES):
        sl = slice(cid * BS, (cid + 1) * BS)
        in_maps.append(
            {
                "x": np.ascontiguousarray(x[sl]),
                "x_RGB": np.ascontiguousarray(xr[sl]),
                "delta": d_b,
            }
        )
    res = run_bass_kernel_spmd(nc, in_maps, core_ids=list(range(N_CORES)), trace=trace)
    out = np.concatenate(
        [np.asarray(r["out"]).astype(np.float32) for r in res.results], axis=0
    )
    return out.reshape(B, C, H, W), res


def kernel(x, x_RGB, delta):
    out, _ = _run(x, x_RGB, delta, trace=False)
    return out
